# revision 1
# baseline (speedup 1.0000x reference)
"""Trainium2 Bass kernel for nn_DynamicKernelSelection (moe_routing).

Strategy
--------
Host (cheap, O(B*C)):
  * Gating in float64 (argmax margins are far above fp32 noise).
  * Samples are paired by identical (expert1, expert2); at most one
    leftover pair can mix experts (pigeonhole) -- the device then runs that
    pair with slot-0's experts and the slot-1 sample is recomputed on the
    host (fp64, tiny fraction of total work).
  * Depthwise conv -> banded Toeplitz lhsT per (channel, kernel column):
    T[h', h] = W[c, dh, dw] at h' = h + (dh-k//2)*dil.  The H-taps become a
    single fp32 128x128 matmul per kernel column; W-shifts are realized by
    accumulating the k_w matmuls into PSUM at shifted column ranges.

Device (SPMD, 8 cores):
  * Work unit = (pair, channel): both samples of a pair are interleaved in
    the free dim as (w, s) so one N=256 matmul covers the pair.
  * Channels are split 128/8: every core runs 16 channels of EVERY pair,
    so per-pair native kernel sizes (3/5 and 7/9/11) give a uniform
    instruction stream across cores AND perfect load balance.
  * Per unit: k1 matmuls -> PSUM -> bias-add evac (VectorE) -> out1 tile
    (DMAed out, reused as stage-2 rhs) -> k2 matmuls -> PSUM -> evac ->
    out2.  All DMAs are [128-partition x contiguous] transfers.
"""

import numpy as np

B, C, H, W = 16, 128, 128, 128
N_CORES = 8
CPC = C // N_CORES           # channels per core per pair (16)
NPAIR = B // 2               # 8 pairs
DIL1, DIL2 = 1, 3
K1S = {0: 3, 1: 5}           # stage-1 expert -> kernel size
K2S = {0: 7, 1: 9, 2: 11}

_PROGS = {}                  # signature -> compiled program


# --------------------------------------------------------------- host math
def _gating(x, aw1, ab1, aw2, ab2):
    pooled = x.astype(np.float64).mean(axis=(2, 3))
    l1 = pooled @ aw1.astype(np.float64).T + ab1.astype(np.float64)
    l2 = pooled @ aw2.astype(np.float64).T + ab2.astype(np.float64)
    return l1.argmax(axis=1), l2.argmax(axis=1)


def _band(wk, dil):
    """wk: [C, k, k] fp32 -> banded lhsT stack [C, H, k*H] fp32."""
    k = wk.shape[-1]
    t = np.zeros((C, H, k, H), np.float32)
    tv = t.transpose(1, 3, 0, 2)  # [h', h, C, dw] view
    c0 = k // 2
    for dh in range(k):
        d = (dh - c0) * dil
        h = np.arange(max(0, -d), H - max(0, d))
        tv[h + d, h] = wk[:, dh, :]
    return np.ascontiguousarray(t.reshape(C, H, k * H))


def _host_conv(x, wk, b, dil):
    """x [C,H,W] fp64, wk [C,k,k], b [C]: same-padded depthwise conv."""
    k = wk.shape[-1]
    c0 = k // 2
    out = np.zeros_like(x)
    for dh in range(k):
        for dw in range(k):
            dh_, dw_ = (dh - c0) * dil, (dw - c0) * dil
            hs = slice(max(0, -dh_), H - max(0, dh_))
            ws = slice(max(0, -dw_), W - max(0, dw_))
            hs2 = slice(max(0, dh_), H - max(0, -dh_))
            ws2 = slice(max(0, dw_), W - max(0, -dw_))
            out[:, hs, ws] += wk[:, dh, dw][:, None, None] * x[:, hs2, ws2]
    return out + b[:, None, None]


def _pair_samples(idx1, idx2):
    """Pair samples by (e1, e2); leftovers paired preferring same e1.
    Returns pairs [(a, b)] and fixes {sample: 'stage2' | 'both'}."""
    from collections import defaultdict
    groups = defaultdict(list)
    for s in range(B):
        groups[(int(idx1[s]), int(idx2[s]))].append(s)
    pairs, singles = [], []
    for key in sorted(groups):
        lst = groups[key]
        while len(lst) >= 2:
            pairs.append((lst.pop(0), lst.pop(0)))
        if lst:
            singles.append(lst[0])
    # pair leftovers: prefer same e1 (then only stage-2 needs a host fix)
    fixes = {}
    while singles:
        a = singles.pop(0)
        bi = next((i for i, s in enumerate(singles)
                   if idx1[s] == idx1[a]), 0)
        b = singles.pop(bi)
        pairs.append((a, b))
        fixes[b] = "stage2" if idx1[b] == idx1[a] else "both"
    return pairs, fixes


# ------------------------------------------------------------ device program
def _build_program(sig):
    """sig: tuple of (k1, k2) per pair."""
    import concourse.tile as tile
    from concourse import bacc, mybir

    dt = mybir.dt.float32
    f16 = mybir.dt.float16
    add = mybir.AluOpType.add
    sub = mybir.AluOpType.subtract
    nc = bacc.Bacc("TRN2", target_bir_lowering=False, debug=False,
                   enable_asserts=False, num_devices=N_CORES)

    xs_d, t1_d, t2_d, b1_d, b2_d, o1_d, o2_d = [], [], [], [], [], [], []
    for p, (k1, k2) in enumerate(sig):
        xs_d.append(nc.dram_tensor(f"x_{p}", [CPC, H, 2 * W], dt,
                                   kind="ExternalInput").ap())
        t1_d.append(nc.dram_tensor(f"t1_{p}", [CPC, H, k1 * H], dt,
                                   kind="ExternalInput").ap())
        t2_d.append(nc.dram_tensor(f"t2_{p}", [CPC, H, 2 * k2 * H], f16,
                                   kind="ExternalInput").ap())
        b1_d.append(nc.dram_tensor(f"b1_{p}", [H, CPC], dt,
                                   kind="ExternalInput").ap())
        b2_d.append(nc.dram_tensor(f"b2_{p}", [H, CPC], dt,
                                   kind="ExternalInput").ap())
        o1_d.append(nc.dram_tensor(f"o1_{p}", [CPC, H, 2 * W], dt,
                                   kind="ExternalOutput").ap())
        o2_d.append(nc.dram_tensor(f"o2_{p}", [CPC, H, 2 * W], dt,
                                   kind="ExternalOutput").ap())

    def conv_mms(psum, tt, src, k, dil):
        c0 = k // 2
        order = [c0] + [dw for dw in range(k) if dw != c0]
        for j, dw in enumerate(order):
            d = (dw - c0) * dil
            a = max(0, -d)
            ln = W - abs(d)
            nc.tensor.matmul(
                out=psum[:, 2 * a:2 * (a + ln)],
                lhsT=tt[:, dw * H:(dw + 1) * H],
                rhs=src[:, 2 * (a + d):2 * (a + d + ln)],
                start=(j == 0), stop=(j == len(order) - 1),
                skip_group_check=True)

    def conv_mms_f16(pm, pl, tt, srch, srcl, k, dil):
        # fp16 hi/lo 3-pass at true scale: pm += Th@xh; pl += Th@xl + Tl@xh
        c0 = k // 2
        order = [c0] + [dw for dw in range(k) if dw != c0]
        for j, dw in enumerate(order):
            d = (dw - c0) * dil
            a = max(0, -d)
            ln = W - abs(d)
            ocols = slice(2 * a, 2 * (a + ln))
            icols = slice(2 * (a + d), 2 * (a + d + ln))
            th = tt[:, (2 * dw) * H:(2 * dw + 1) * H]
            tl = tt[:, (2 * dw + 1) * H:(2 * dw + 2) * H]
            last = j == len(order) - 1
            nc.tensor.matmul(out=pm[:, ocols], lhsT=th, rhs=srch[:, icols],
                             start=(j == 0), stop=last,
                             skip_group_check=True)
            nc.tensor.matmul(out=pl[:, ocols], lhsT=th, rhs=srcl[:, icols],
                             start=(j == 0), stop=False,
                             skip_group_check=True)
            nc.tensor.matmul(out=pl[:, ocols], lhsT=tl, rhs=srch[:, icols],
                             start=False, stop=last,
                             skip_group_check=True)

    with tile.TileContext(nc) as tc:
        with (tc.tile_pool(name="xp", bufs=6) as xp,
              tc.tile_pool(name="o1p", bufs=4) as o1p,
              tc.tile_pool(name="o2p", bufs=4) as o2p,
              tc.tile_pool(name="t1p", bufs=4) as t1p,
              tc.tile_pool(name="t2p", bufs=4) as t2p,
              tc.tile_pool(name="bp", bufs=2) as bp,
              tc.tile_pool(name="tm", bufs=4) as tm,
              tc.tile_pool(name="ps", bufs=8, space="PSUM") as ps):
            # software-pipelined two units deep: PE never waits on the
            # DVE evac + hi/lo split chain of the previous units.
            pend = []

            def emit_stage2(st):
                p, u, k2, o1h, o1l, b2t = st
                t2t = t2p.tile([128, 2 * k2 * H], f16, tag="t2")
                nc.sync.dma_start(out=t2t[:], in_=t2_d[p][u])
                pm2 = ps.tile([128, 2 * W], dt, tag="ps")
                pl2 = ps.tile([128, 2 * W], dt, tag="ps")
                conv_mms_f16(pm2, pl2, t2t, o1h, o1l, k2, DIL2)
                o2c = o2p.tile([128, 2 * W], dt, tag="o2")
                tmp = tm.tile([128, 2 * W], dt, tag="tmp")
                nc.vector.tensor_copy(out=tmp[:], in_=pl2[:])
                nc.vector.scalar_tensor_tensor(
                    out=o2c[:], in0=pm2[:], scalar=b2t[:, u:u + 1],
                    in1=tmp[:], op0=add, op1=add)
                nc.sync.dma_start(out=o2_d[p][u], in_=o2c[:])

            for p, (k1, k2) in enumerate(sig):
                b1t = bp.tile([128, CPC], dt, tag="b1")
                nc.sync.dma_start(out=b1t[:], in_=b1_d[p])
                b2t = bp.tile([128, CPC], dt, tag="b2")
                nc.sync.dma_start(out=b2t[:], in_=b2_d[p])
                for u in range(CPC):
                    xc = xp.tile([128, 2 * W], dt, tag="x")
                    nc.sync.dma_start(out=xc[:], in_=xs_d[p][u])
                    t1t = t1p.tile([128, k1 * H], dt, tag="t1")
                    nc.sync.dma_start(out=t1t[:], in_=t1_d[p][u])
                    p1 = ps.tile([128, 2 * W], dt, tag="ps")
                    conv_mms(p1, t1t, xc, k1, DIL1)
                    o1c = o1p.tile([128, 2 * W], dt, tag="o1")
                    nc.vector.tensor_scalar(out=o1c[:], in0=p1[:],
                                            scalar1=b1t[:, u:u + 1],
                                            scalar2=None, op0=add)
                    nc.sync.dma_start(out=o1_d[p][u], in_=o1c[:])
                    # split o1 into fp16 hi + lo for the fp16 stage 2
                    o1h = o1p.tile([128, 2 * W], f16, tag="o1h")
                    nc.vector.tensor_copy(out=o1h[:], in_=o1c[:])
                    o1l = o1p.tile([128, 2 * W], f16, tag="o1l")
                    nc.vector.tensor_tensor(out=o1l[:], in0=o1c[:],
                                            in1=o1h[:], op=sub)
                    pend.append((p, u, k2, o1h, o1l, b2t))
                    if len(pend) > 3:
                        emit_stage2(pend.pop(0))
            while pend:
                emit_stage2(pend.pop(0))
    nc.compile()
    return nc


# ------------------------------------------------------------------- driver
def kernel(x, aw1, ab1, aw2, ab2, w1_3, b1_3, w1_5, b1_5,
           w2_7, b2_7, w2_9, b2_9, w2_11, b2_11):
    from concourse.bass_utils import run_bass_kernel_spmd

    x = np.ascontiguousarray(np.asarray(x, dtype=np.float32))
    assert x.shape == (B, C, H, W)

    idx1, idx2 = _gating(np.asarray(x), np.asarray(aw1), np.asarray(ab1),
                         np.asarray(aw2), np.asarray(ab2))
    pairs, fixes = _pair_samples(idx1, idx2)

    w1e = [np.ascontiguousarray(np.asarray(w, np.float32)[:, 0])
           for w in (w1_3, w1_5)]
    w2e = [np.ascontiguousarray(np.asarray(w, np.float32)[:, 0])
           for w in (w2_7, w2_9, w2_11)]
    b1e = [np.asarray(b, np.float32) for b in (b1_3, b1_5)]
    b2e = [np.asarray(b, np.float32) for b in (b2_7, b2_9, b2_11)]

    # per-pair experts = slot-0's selection
    pe1 = [int(idx1[a]) for a, _ in pairs]
    pe2 = [int(idx2[a]) for a, _ in pairs]
    sig = tuple((K1S[e1], K2S[e2]) for e1, e2 in zip(pe1, pe2))

    if sig not in _PROGS:
        _PROGS[sig] = _build_program(sig)
    nc = _PROGS[sig]

    def _band_hilo(wk, dil):
        k = wk.shape[-1]
        band = _band(wk, dil).reshape(C, H, k, H)
        hi = band.astype(np.float16)
        lo = (band - hi.astype(np.float32)).astype(np.float16)
        return np.ascontiguousarray(
            np.stack([hi, lo], axis=3).reshape(C, H, 2 * k * H))

    # stage-1 lhsT fp32; stage-2 lhsT fp16 hi/lo per distinct expert
    t1b = {e: _band(w1e[e], DIL1) for e in set(pe1)}
    t2b = {e: _band_hilo(w2e[e], DIL2) for e in set(pe2)}

    # per-pair interleaved x: [C, H, W, 2] -> [C, H, 2W]
    xpair = []
    for a, b in pairs:
        xi = np.stack([x[a], x[b]], axis=-1).reshape(C, H, 2 * W)
        xpair.append(xi)

    in_maps = []
    for core in range(N_CORES):
        cs = slice(core * CPC, (core + 1) * CPC)
        m = {}
        for p, (e1, e2) in enumerate(zip(pe1, pe2)):
            m[f"x_{p}"] = xpair[p][cs]
            m[f"t1_{p}"] = t1b[e1][cs]
            m[f"t2_{p}"] = t2b[e2][cs]
            m[f"b1_{p}"] = np.ascontiguousarray(
                np.broadcast_to(b1e[e1][None, cs], (H, CPC)))
            m[f"b2_{p}"] = np.ascontiguousarray(
                np.broadcast_to(b2e[e2][None, cs], (H, CPC)))
        in_maps.append(m)

    res = run_bass_kernel_spmd(nc, in_maps, list(range(N_CORES)))

    out1 = np.empty((B, C, H, W), np.float32)
    out2 = np.empty((B, C, H, W), np.float32)
    for core in range(N_CORES):
        cs = slice(core * CPC, (core + 1) * CPC)
        r = res.results[core]
        for p, (a, b) in enumerate(pairs):
            o1 = r[f"o1_{p}"].reshape(CPC, H, W, 2)
            o2 = r[f"o2_{p}"].reshape(CPC, H, W, 2)
            out1[a, cs], out1[b, cs] = o1[..., 0], o1[..., 1]
            out2[a, cs], out2[b, cs] = o2[..., 0], o2[..., 1]

    # host fix-up for mixed pairs (at most 2 samples)
    for s, kind in fixes.items():
        e1, e2 = int(idx1[s]), int(idx2[s])
        if kind == "both":
            o1 = _host_conv(x[s].astype(np.float64), w1e[e1].astype(np.float64),
                            b1e[e1].astype(np.float64), DIL1)
            out1[s] = o1.astype(np.float32)
        else:
            o1 = out1[s].astype(np.float64)
        out2[s] = _host_conv(o1, w2e[e2].astype(np.float64),
                             b2e[e2].astype(np.float64), DIL2).astype(np.float32)
    return out1, out2



# revision 4
# speedup vs baseline: 1.9247x; 1.9247x over previous
"""Trainium2 Bass kernel for nn_DynamicKernelSelection (moe_routing).

Strategy
--------
Host (cheap, O(B*C)):
  * Gating in float64 (argmax margins ~4e-4, far above fp32 noise).
  * Samples are grouped by identical (expert1, expert2) signature and the
    groups are cut into chunks of <= 4 samples (PSUM bank = 512 fp32 cols).
    No mixed chunks, so no host fix-up convs are needed.
  * Depthwise conv -> banded Toeplitz lhsT per (channel, kernel column):
    T[h', h] = W[c, dh, dw] at h' = h + (dh-k//2)*dil.  The H-taps become a
    single fp16 128x128 matmul per kernel column; W-shifts are realized by
    accumulating the k_w matmuls into PSUM at shifted column ranges.

Device (SPMD, 8 cores):
  * Channels split 128/8: every core runs 16 channels of EVERY chunk, so
    the instruction stream is uniform across cores (perfect balance).
  * Channel-outer loop: the per-channel Toeplitz matrices (all experts)
    are DMAed once per channel and reused by every chunk -> ~4x less
    weight traffic than per-(pair,channel) streaming.
  * Everything is single-pass fp16 on the PE (1 cycle/column): tolerance
    is 2e-2 while fp16 single-pass lands ~1e-3.  x, T, o1, o2 all fp16
    (halves DMA); PSUM accumulates in fp32.
  * Per (channel, chunk): k1 matmuls -> PSUM -> evac (DVE) -> o1 fp16 tile
    (DMAed out, reused as stage-2 rhs) -> k2 matmuls -> PSUM -> evac
    (ACT) -> o2.  Stage-2 is emitted two chunks behind stage-1 so the PE
    never waits on an evac.
"""

import numpy as np

B, C, H, W = 16, 128, 128, 128
N_CORES = 8
CPC = C // N_CORES           # channels per core (16)
DIL1, DIL2 = 1, 3
K1S = {0: 3, 1: 5}           # stage-1 expert -> kernel size
K2S = {0: 7, 1: 9, 2: 11}
NMAX = 4                     # samples per chunk (4*W = 512 = one PSUM bank)

_PROGS = {}                  # signature -> compiled program


# --------------------------------------------------------------- host math
def _gating(x, aw1, ab1, aw2, ab2):
    pooled = x.astype(np.float64).mean(axis=(2, 3))
    l1 = pooled @ aw1.astype(np.float64).T + ab1.astype(np.float64)
    l2 = pooled @ aw2.astype(np.float64).T + ab2.astype(np.float64)
    return l1.argmax(axis=1), l2.argmax(axis=1)


def _band(wk, dil):
    """wk: [C, k, k] fp32 -> banded lhsT stack [C, H, k*H] fp16."""
    k = wk.shape[-1]
    t = np.zeros((C, H, k, H), np.float32)
    tv = t.transpose(1, 3, 0, 2)  # [h', h, C, dw] view
    c0 = k // 2
    for dh in range(k):
        d = (dh - c0) * dil
        h = np.arange(max(0, -d), H - max(0, d))
        tv[h + d, h] = wk[:, dh, :]
    return np.ascontiguousarray(t.reshape(C, H, k * H).astype(np.float16))


def _chunk_samples(idx1, idx2):
    """Group samples by (e1, e2); cut groups into chunks of <= NMAX.
    Returns [(e1, e2, [samples])]."""
    from collections import defaultdict
    groups = defaultdict(list)
    for s in range(B):
        groups[(int(idx1[s]), int(idx2[s]))].append(s)
    chunks = []
    for key in sorted(groups):
        lst = groups[key]
        for i in range(0, len(lst), NMAX):
            chunks.append((key[0], key[1], lst[i:i + NMAX]))
    return chunks


# ------------------------------------------------------------ device program
def _build_program(sig):
    """sig: tuple of (k1, k2, n) per chunk."""
    import concourse.tile as tile
    from concourse import bacc, mybir

    f16 = mybir.dt.float16
    f32 = mybir.dt.float32
    copy_f = mybir.ActivationFunctionType.Copy
    nc = bacc.Bacc("TRN2", target_bir_lowering=False, debug=False,
                   enable_asserts=False, num_devices=N_CORES)

    k1s = sorted({k1 for k1, _, _ in sig})
    k2s = sorted({k2 for _, k2, _ in sig})
    t1_d = {k: nc.dram_tensor(f"t1_{k}", [CPC, H, k * H], f16,
                              kind="ExternalInput").ap() for k in k1s}
    t2_d = {k: nc.dram_tensor(f"t2_{k}", [CPC, H, k * H], f16,
                              kind="ExternalInput").ap() for k in k2s}
    x_d, o1_d, o2_d = [], [], []
    for g, (k1, k2, n) in enumerate(sig):
        x_d.append(nc.dram_tensor(f"x_{g}", [CPC, H, n * W], f16,
                                  kind="ExternalInput").ap())
        o1_d.append(nc.dram_tensor(f"o1_{g}", [CPC, H, n * W], f16,
                                   kind="ExternalOutput").ap())
        o2_d.append(nc.dram_tensor(f"o2_{g}", [CPC, H, n * W], f16,
                                   kind="ExternalOutput").ap())

    def conv_mms(psum, tt, src, k, dil, n):
        c0 = k // 2
        order = [c0] + [dw for dw in range(k) if dw != c0]
        for j, dw in enumerate(order):
            d = (dw - c0) * dil
            a = max(0, -d)
            ln = W - abs(d)
            nc.tensor.matmul(
                out=psum[:, n * a:n * (a + ln)],
                lhsT=tt[:, dw * H:(dw + 1) * H],
                rhs=src[:, n * (a + d):n * (a + d + ln)],
                start=(j == 0), stop=(j == len(order) - 1),
                skip_group_check=True)

    with tile.TileContext(nc) as tc:
        with (tc.tile_pool(name="xp", bufs=4) as xp,
              tc.tile_pool(name="o1p", bufs=4) as o1p,
              tc.tile_pool(name="o2p", bufs=4) as o2p,
              tc.tile_pool(name="t1p", bufs=2) as t1p,
              tc.tile_pool(name="t2p", bufs=2) as t2p,
              tc.tile_pool(name="ps", bufs=6, space="PSUM") as ps):
            pend = []

            def emit_stage2(st):
                g, u, k2, n, o1c, t2t = st
                p2 = ps.tile([128, n * W], f32, tag="ps")
                conv_mms(p2, t2t[k2], o1c, k2, DIL2, n)
                o2c = o2p.tile([128, n * W], f16, tag="o2")
                nc.scalar.activation(out=o2c[:], in_=p2[:], func=copy_f)
                nc.sync.dma_start(out=o2_d[g][u], in_=o2c[:])

            for u in range(CPC):
                t1t = {}
                for k in k1s:
                    t1t[k] = t1p.tile([128, k * H], f16, tag=f"t1_{k}",
                                      name=f"t1t_{k}_{u}")
                    nc.sync.dma_start(out=t1t[k][:], in_=t1_d[k][u])
                t2t = {}
                for k in k2s:
                    t2t[k] = t2p.tile([128, k * H], f16, tag=f"t2_{k}",
                                      name=f"t2t_{k}_{u}")
                    nc.sync.dma_start(out=t2t[k][:], in_=t2_d[k][u])
                for g, (k1, k2, n) in enumerate(sig):
                    xc = xp.tile([128, n * W], f16, tag="x")
                    nc.sync.dma_start(out=xc[:], in_=x_d[g][u])
                    p1 = ps.tile([128, n * W], f32, tag="ps")
                    conv_mms(p1, t1t[k1], xc, k1, DIL1, n)
                    o1c = o1p.tile([128, n * W], f16, tag="o1")
                    nc.vector.tensor_copy(out=o1c[:], in_=p1[:])
                    nc.sync.dma_start(out=o1_d[g][u], in_=o1c[:])
                    pend.append((g, u, k2, n, o1c, t2t))
                    if len(pend) > 2:
                        emit_stage2(pend.pop(0))
            while pend:
                emit_stage2(pend.pop(0))
    nc.compile()
    return nc


# ------------------------------------------------------------------- driver
def kernel(x, aw1, ab1, aw2, ab2, w1_3, b1_3, w1_5, b1_5,
           w2_7, b2_7, w2_9, b2_9, w2_11, b2_11):
    from concourse.bass_utils import run_bass_kernel_spmd

    x = np.ascontiguousarray(np.asarray(x, dtype=np.float32))
    assert x.shape == (B, C, H, W)

    idx1, idx2 = _gating(np.asarray(x), np.asarray(aw1), np.asarray(ab1),
                         np.asarray(aw2), np.asarray(ab2))
    chunks = _chunk_samples(idx1, idx2)
    sig = tuple((K1S[e1], K2S[e2], len(ss)) for e1, e2, ss in chunks)

    # biases are zero in this problem family; fall back to host add if not
    b1e = [np.asarray(b, np.float32) for b in (b1_3, b1_5)]
    b2e = [np.asarray(b, np.float32) for b in (b2_7, b2_9, b2_11)]

    if sig not in _PROGS:
        _PROGS[sig] = _build_program(sig)
    nc = _PROGS[sig]

    w1e = [np.asarray(w, np.float32)[:, 0] for w in (w1_3, w1_5)]
    w2e = [np.asarray(w, np.float32)[:, 0] for w in (w2_7, w2_9, w2_11)]
    e1_used = sorted({e1 for e1, _, _ in chunks})
    e2_used = sorted({e2 for _, e2, _ in chunks})
    t1b = {K1S[e]: _band(w1e[e], DIL1) for e in e1_used}
    t2b = {K2S[e]: _band(w2e[e], DIL2) for e in e2_used}

    # per-chunk interleaved x: [C, H, W, n] -> [C, H, n*W] fp16
    xg = []
    for e1, e2, ss in chunks:
        xi = np.stack([x[s] for s in ss], axis=-1)
        xg.append(np.ascontiguousarray(
            xi.reshape(C, H, len(ss) * W).astype(np.float16)))

    in_maps = []
    for core in range(N_CORES):
        cs = slice(core * CPC, (core + 1) * CPC)
        m = {}
        for k, tb in t1b.items():
            m[f"t1_{k}"] = tb[cs]
        for k, tb in t2b.items():
            m[f"t2_{k}"] = tb[cs]
        for g in range(len(chunks)):
            m[f"x_{g}"] = xg[g][cs]
        in_maps.append(m)

    res = run_bass_kernel_spmd(nc, in_maps, list(range(N_CORES)))

    out1 = np.empty((B, C, H, W), np.float32)
    out2 = np.empty((B, C, H, W), np.float32)
    for core in range(N_CORES):
        cs = slice(core * CPC, (core + 1) * CPC)
        r = res.results[core]
        for g, (e1, e2, ss) in enumerate(chunks):
            n = len(ss)
            o1 = r[f"o1_{g}"].reshape(CPC, H, W, n).astype(np.float32)
            o2 = r[f"o2_{g}"].reshape(CPC, H, W, n).astype(np.float32)
            for j, s in enumerate(ss):
                out1[s, cs] = o1[..., j]
                out2[s, cs] = o2[..., j]

    # host bias add (zero in this problem family; kept for generality).
    # out1 += b1; the b1 constant image also feeds conv2, whose border
    # truncation makes the correction conv2(b1*ones) position-dependent.
    if any(b.any() for b in b1e) or any(b.any() for b in b2e):
        corr = {}
        for s in range(B):
            e1, e2 = int(idx1[s]), int(idx2[s])
            b1, b2 = b1e[e1], b2e[e2]
            out1[s] += b1[:, None, None]
            if (e1, e2) not in corr:
                img = np.broadcast_to(b1[:, None, None].astype(np.float64),
                                      (C, H, W))
                corr[(e1, e2)] = _host_conv_nobias(
                    img, w2e[e2].astype(np.float64), DIL2)
            out2[s] += (corr[(e1, e2)] + b2[:, None, None]).astype(np.float32)
    return out1, out2


def _host_conv_nobias(x, wk, dil):
    """x [C,H,W] fp64, wk [C,k,k]: same-padded depthwise conv, no bias."""
    k = wk.shape[-1]
    c0 = k // 2
    out = np.zeros_like(x)
    for dh in range(k):
        for dw in range(k):
            dh_, dw_ = (dh - c0) * dil, (dw - c0) * dil
            hs = slice(max(0, -dh_), H - max(0, dh_))
            ws = slice(max(0, -dw_), W - max(0, dw_))
            hs2 = slice(max(0, dh_), H - max(0, -dh_))
            ws2 = slice(max(0, dw_), W - max(0, -dw_))
            out[:, hs, ws] += wk[:, dh, dw][:, None, None] * x[:, hs2, ws2]
    return out


# revision 6
# speedup vs baseline: 3.1019x; 1.6117x over previous
"""Trainium2 Bass kernel for nn_DynamicKernelSelection (moe_routing).

Strategy
--------
Host (cheap, O(B*C)):
  * Gating in float64 (argmax margins ~4e-4, far above fp32 noise).
  * Samples are grouped by identical (expert1, expert2) signature and the
    groups are cut into chunks of <= 4 samples (PSUM bank = 512 fp32 cols).
    No mixed chunks, so no host fix-up convs are needed.
  * Depthwise conv -> banded Toeplitz lhsT per (channel, kernel column):
    T[h', h] = W[c, dh, dw] at h' = h + (dh-k//2)*dil.  The H-taps become a
    single fp16 128x128 matmul per kernel column; W-shifts are realized by
    accumulating the k_w matmuls into PSUM at shifted column ranges.

Device (SPMD, 8 cores):
  * Channels split 128/8: every core runs 16 channels of EVERY chunk, so
    the instruction stream is uniform across cores (perfect balance).
  * Channel-outer loop: per channel ONE batched DMA each for (all experts'
    Toeplitz weights), (all 16 samples of x), (o1 out), (o2 out).  A
    dma_start costs ~630ns of sequencer issue time, so 4 big DMAs/channel
    (64/core) instead of ~416 keeps the DGE queues off the critical path
    (v2 measured Sync 88% busy / PE 62% busy at 416 DMAs).
  * Everything is single-pass fp16 on the PE (1 cycle/column): tolerance
    is 2e-2 while fp16 single-pass lands ~6e-4.  x, T, o1, o2 all fp16
    (halves DMA); PSUM accumulates in fp32.
  * Per (channel, chunk): k1 matmuls -> PSUM -> evac (DVE) -> o1 slice
    (stage-2 rhs) -> k2 matmuls -> PSUM -> evac (ACT) -> o2 slice.
    Stage-2 is emitted two chunks behind stage-1 so the PE never waits
    on an evac.  Input DMAs issue on SP, output DMAs on ACT (both HWDGE).
"""

import numpy as np

B, C, H, W = 16, 128, 128, 128
N_CORES = 8
CPC = C // N_CORES           # channels per core (16)
DIL1, DIL2 = 1, 3
K1S = {0: 3, 1: 5}           # stage-1 expert -> kernel size
K2S = {0: 7, 1: 9, 2: 11}
NMAX = 4                     # samples per chunk (4*W = 512 = one PSUM bank)

_PROGS = {}                  # signature -> compiled program


# --------------------------------------------------------------- host math
def _gating(x, aw1, ab1, aw2, ab2):
    pooled = x.astype(np.float64).mean(axis=(2, 3))
    l1 = pooled @ aw1.astype(np.float64).T + ab1.astype(np.float64)
    l2 = pooled @ aw2.astype(np.float64).T + ab2.astype(np.float64)
    return l1.argmax(axis=1), l2.argmax(axis=1)


def _band(wk, dil):
    """wk: [C, k, k] fp32 -> banded lhsT stack [C, H, k*H] fp16."""
    k = wk.shape[-1]
    t = np.zeros((C, H, k, H), np.float32)
    tv = t.transpose(1, 3, 0, 2)  # [h', h, C, dw] view
    c0 = k // 2
    for dh in range(k):
        d = (dh - c0) * dil
        h = np.arange(max(0, -d), H - max(0, d))
        tv[h + d, h] = wk[:, dh, :]
    return np.ascontiguousarray(t.reshape(C, H, k * H).astype(np.float16))


def _chunk_samples(idx1, idx2):
    """Group samples by (e1, e2); cut groups into chunks of <= NMAX.
    Returns [(e1, e2, [samples])]."""
    from collections import defaultdict
    groups = defaultdict(list)
    for s in range(B):
        groups[(int(idx1[s]), int(idx2[s]))].append(s)
    chunks = []
    for key in sorted(groups):
        lst = groups[key]
        for i in range(0, len(lst), NMAX):
            chunks.append((key[0], key[1], lst[i:i + NMAX]))
    return chunks


# ------------------------------------------------------------ device program
def _build_program(sig):
    """sig: tuple of (k1, k2, n) per chunk."""
    import concourse.tile as tile
    from concourse import bacc, mybir

    f16 = mybir.dt.float16
    f32 = mybir.dt.float32
    copy_f = mybir.ActivationFunctionType.Copy
    nc = bacc.Bacc("TRN2", target_bir_lowering=False, debug=False,
                   enable_asserts=False, num_devices=N_CORES)

    k1s = sorted({k1 for k1, _, _ in sig})
    k2s = sorted({k2 for _, k2, _ in sig})
    # column offset of each expert's Toeplitz block inside the fused T tile
    toff = {}
    tcols = 0
    for k in k1s:
        toff[("s1", k)] = tcols
        tcols += k * H
    for k in k2s:
        toff[("s2", k)] = tcols
        tcols += k * H
    # column offset of each chunk inside the fused x/o1/o2 tiles
    goff = []
    xcols = 0
    for k1, k2, n in sig:
        goff.append(xcols)
        xcols += n * W

    t_d = nc.dram_tensor("t", [CPC, H, tcols], f16, kind="ExternalInput").ap()
    x_d = nc.dram_tensor("x", [CPC, H, xcols], f16, kind="ExternalInput").ap()
    o1_d = nc.dram_tensor("o1", [CPC, H, xcols], f16,
                          kind="ExternalOutput").ap()
    o2_d = nc.dram_tensor("o2", [CPC, H, xcols], f16,
                          kind="ExternalOutput").ap()

    def conv_mms(psum, tt, t0, src, s0, k, dil, n):
        """psum[:, :n*W] += conv(src at col offset s0) with T block at t0."""
        c0 = k // 2
        order = [c0] + [dw for dw in range(k) if dw != c0]
        for j, dw in enumerate(order):
            d = (dw - c0) * dil
            a = max(0, -d)
            ln = W - abs(d)
            nc.tensor.matmul(
                out=psum[:, n * a:n * (a + ln)],
                lhsT=tt[:, t0 + dw * H:t0 + (dw + 1) * H],
                rhs=src[:, s0 + n * (a + d):s0 + n * (a + d + ln)],
                start=(j == 0), stop=(j == len(order) - 1),
                skip_group_check=True)

    with tile.TileContext(nc) as tc:
        with (tc.tile_pool(name="xp", bufs=3) as xp,
              tc.tile_pool(name="o1p", bufs=3) as o1p,
              tc.tile_pool(name="o2p", bufs=3) as o2p,
              tc.tile_pool(name="tp", bufs=2) as tp,
              tc.tile_pool(name="ps", bufs=6, space="PSUM") as ps):
            pend = []
            s2_left = {}          # channel -> stage-2 chunks not yet emitted

            def emit_stage2(st):
                g, u, k2, n, o1c, o2c, tt = st
                p2 = ps.tile([128, n * W], f32, tag="ps")
                conv_mms(p2, tt, toff[("s2", k2)], o1c, goff[g], k2, DIL2, n)
                nc.scalar.activation(out=o2c[:, goff[g]:goff[g] + n * W],
                                     in_=p2[:], func=copy_f)
                s2_left[u] -= 1
                if s2_left[u] == 0:
                    nc.scalar.dma_start(out=o2_d[u], in_=o2c[:])

            for u in range(CPC):
                tt = tp.tile([128, tcols], f16, tag="t", name=f"tt_{u}")
                nc.sync.dma_start(out=tt[:], in_=t_d[u])
                xc = xp.tile([128, xcols], f16, tag="x", name=f"xc_{u}")
                nc.sync.dma_start(out=xc[:], in_=x_d[u])
                o1c = o1p.tile([128, xcols], f16, tag="o1", name=f"o1c_{u}")
                o2c = o2p.tile([128, xcols], f16, tag="o2", name=f"o2c_{u}")
                s2_left[u] = len(sig)
                for g, (k1, k2, n) in enumerate(sig):
                    p1 = ps.tile([128, n * W], f32, tag="ps")
                    conv_mms(p1, tt, toff[("s1", k1)], xc, goff[g],
                             k1, DIL1, n)
                    nc.vector.tensor_copy(
                        out=o1c[:, goff[g]:goff[g] + n * W], in_=p1[:])
                    pend.append((g, u, k2, n, o1c, o2c, tt))
                    if len(pend) > 2:
                        emit_stage2(pend.pop(0))
                    if g == len(sig) - 1:
                        nc.scalar.dma_start(out=o1_d[u], in_=o1c[:])
            while pend:
                emit_stage2(pend.pop(0))
    nc.compile()
    return nc


# ------------------------------------------------------------------- driver
def kernel(x, aw1, ab1, aw2, ab2, w1_3, b1_3, w1_5, b1_5,
           w2_7, b2_7, w2_9, b2_9, w2_11, b2_11):
    from concourse.bass_utils import run_bass_kernel_spmd

    x = np.ascontiguousarray(np.asarray(x, dtype=np.float32))
    assert x.shape == (B, C, H, W)

    idx1, idx2 = _gating(np.asarray(x), np.asarray(aw1), np.asarray(ab1),
                         np.asarray(aw2), np.asarray(ab2))
    chunks = _chunk_samples(idx1, idx2)
    sig = tuple((K1S[e1], K2S[e2], len(ss)) for e1, e2, ss in chunks)

    b1e = [np.asarray(b, np.float32) for b in (b1_3, b1_5)]
    b2e = [np.asarray(b, np.float32) for b in (b2_7, b2_9, b2_11)]

    if sig not in _PROGS:
        _PROGS[sig] = _build_program(sig)
    nc = _PROGS[sig]

    w1e = [np.asarray(w, np.float32)[:, 0] for w in (w1_3, w1_5)]
    w2e = [np.asarray(w, np.float32)[:, 0] for w in (w2_7, w2_9, w2_11)]
    k1s = sorted({K1S[e1] for e1, _, _ in chunks})
    k2s = sorted({K2S[e2] for _, e2, _ in chunks})
    e1_of = {K1S[e]: e for e in range(2)}
    e2_of = {K2S[e]: e for e in range(3)}
    tparts = [_band(w1e[e1_of[k]], DIL1) for k in k1s]
    tparts += [_band(w2e[e2_of[k]], DIL2) for k in k2s]
    tall = np.concatenate(tparts, axis=2)  # [C, H, tcols] fp16

    # fused interleaved x: per chunk [C, H, W, n] -> concat -> [C, H, xcols]
    xg = []
    for e1, e2, ss in chunks:
        xi = np.stack([x[s] for s in ss], axis=-1)
        xg.append(xi.reshape(C, H, len(ss) * W))
    xall = np.concatenate(xg, axis=2).astype(np.float16)

    in_maps = []
    for core in range(N_CORES):
        cs = slice(core * CPC, (core + 1) * CPC)
        in_maps.append({"t": tall[cs], "x": np.ascontiguousarray(xall[cs])})

    res = run_bass_kernel_spmd(nc, in_maps, list(range(N_CORES)))

    out1 = np.empty((B, C, H, W), np.float32)
    out2 = np.empty((B, C, H, W), np.float32)
    goff = np.cumsum([0] + [len(ss) * W for _, _, ss in chunks])
    for core in range(N_CORES):
        cs = slice(core * CPC, (core + 1) * CPC)
        r = res.results[core]
        o1 = r["o1"].astype(np.float32)
        o2 = r["o2"].astype(np.float32)
        for g, (e1, e2, ss) in enumerate(chunks):
            n = len(ss)
            c1 = o1[:, :, goff[g]:goff[g + 1]].reshape(CPC, H, W, n)
            c2 = o2[:, :, goff[g]:goff[g + 1]].reshape(CPC, H, W, n)
            for j, s in enumerate(ss):
                out1[s, cs] = c1[..., j]
                out2[s, cs] = c2[..., j]

    # host bias add (zero in this problem family; kept for generality).
    if any(b.any() for b in b1e) or any(b.any() for b in b2e):
        corr = {}
        for s in range(B):
            e1, e2 = int(idx1[s]), int(idx2[s])
            b1, b2 = b1e[e1], b2e[e2]
            out1[s] += b1[:, None, None]
            if (e1, e2) not in corr:
                img = np.broadcast_to(b1[:, None, None].astype(np.float64),
                                      (C, H, W))
                corr[(e1, e2)] = _host_conv_nobias(
                    img, w2e[e2].astype(np.float64), DIL2)
            out2[s] += (corr[(e1, e2)] + b2[:, None, None]).astype(np.float32)
    return out1, out2


def _host_conv_nobias(x, wk, dil):
    """x [C,H,W] fp64, wk [C,k,k]: same-padded depthwise conv, no bias."""
    k = wk.shape[-1]
    c0 = k // 2
    out = np.zeros_like(x)
    for dh in range(k):
        for dw in range(k):
            dh_, dw_ = (dh - c0) * dil, (dw - c0) * dil
            hs = slice(max(0, -dh_), H - max(0, dh_))
            ws = slice(max(0, -dw_), W - max(0, dw_))
            hs2 = slice(max(0, dh_), H - max(0, -dh_))
            ws2 = slice(max(0, dw_), W - max(0, -dw_))
            out[:, hs, ws] += wk[:, dh, dw][:, None, None] * x[:, hs2, ws2]
    return out

# revision 8
# speedup vs baseline: 3.1242x; 1.0072x over previous
"""Trainium2 Bass kernel for nn_DynamicKernelSelection (moe_routing).

Strategy
--------
Host (cheap, O(B*C)):
  * Gating in float64 (argmax margins ~4e-4, far above fp32 noise).
  * Samples are grouped by identical (expert1, expert2) signature and the
    groups are cut into chunks of <= 4 samples (PSUM bank = 512 fp32 cols).
    No mixed chunks, so no host fix-up convs are needed.
  * Depthwise conv -> banded Toeplitz lhsT per (channel, kernel column):
    T[h', h] = W[c, dh, dw] at h' = h + (dh-k//2)*dil.  The H-taps become a
    single fp16 128x128 matmul per kernel column; W-shifts are realized by
    accumulating the k_w matmuls into PSUM at shifted column ranges.

Device (SPMD, 8 cores):
  * Channels split 128/8: every core runs 16 channels of EVERY chunk, so
    the instruction stream is uniform across cores (perfect balance).
  * Channel-outer loop: per channel ONE batched DMA each for (all experts'
    Toeplitz weights), (all 16 samples of x), (o1 out), (o2 out).  A
    dma_start costs ~630ns of sequencer issue time, so 4 big DMAs/channel
    (64/core) instead of ~416 keeps the DGE queues off the critical path
    (v2 measured Sync 88% busy / PE 62% busy at 416 DMAs).
  * Everything is single-pass fp16 on the PE (1 cycle/column): tolerance
    is 2e-2 while fp16 single-pass lands ~6e-4.  x, T, o1, o2 all fp16
    (halves DMA); PSUM accumulates in fp32.
  * Per (channel, chunk): k1 matmuls -> PSUM -> evac (DVE) -> o1 slice
    (stage-2 rhs) -> k2 matmuls -> PSUM -> evac (ACT) -> o2 slice.
    Stage-2 is emitted two chunks behind stage-1 so the PE never waits
    on an evac.  Input DMAs issue on SP, output DMAs on ACT (both HWDGE).
"""

import numpy as np

B, C, H, W = 16, 128, 128, 128
N_CORES = 8
CPC = C // N_CORES           # channels per core (16)
DIL1, DIL2 = 1, 3
K1S = {0: 3, 1: 5}           # stage-1 expert -> kernel size
K2S = {0: 7, 1: 9, 2: 11}
NMAX = 4                     # samples per chunk (4*W = 512 = one PSUM bank)

_PROGS = {}                  # signature -> compiled program


# --------------------------------------------------------------- host math
def _gating(x, aw1, ab1, aw2, ab2):
    pooled = x.astype(np.float64).mean(axis=(2, 3))
    l1 = pooled @ aw1.astype(np.float64).T + ab1.astype(np.float64)
    l2 = pooled @ aw2.astype(np.float64).T + ab2.astype(np.float64)
    return l1.argmax(axis=1), l2.argmax(axis=1)


def _band(wk, dil):
    """wk: [C, k, k] fp32 -> banded lhsT stack [C, H, k*H] fp16."""
    k = wk.shape[-1]
    t = np.zeros((C, H, k, H), np.float32)
    tv = t.transpose(1, 3, 0, 2)  # [h', h, C, dw] view
    c0 = k // 2
    for dh in range(k):
        d = (dh - c0) * dil
        h = np.arange(max(0, -d), H - max(0, d))
        tv[h + d, h] = wk[:, dh, :]
    return np.ascontiguousarray(t.reshape(C, H, k * H).astype(np.float16))


def _chunk_samples(idx1, idx2):
    """Group samples by (e1, e2); cut groups into chunks of <= NMAX.
    Returns [(e1, e2, [samples])]."""
    from collections import defaultdict
    groups = defaultdict(list)
    for s in range(B):
        groups[(int(idx1[s]), int(idx2[s]))].append(s)
    chunks = []
    for key in sorted(groups):
        lst = groups[key]
        for i in range(0, len(lst), NMAX):
            chunks.append((key[0], key[1], lst[i:i + NMAX]))
    # big chunks first: the final stage-2 group (kernel tail) is smallest
    chunks.sort(key=lambda c: -len(c[2]))
    return chunks


# ------------------------------------------------------------ device program
def _build_program(sig):
    """sig: tuple of (k1, k2, n) per chunk."""
    import concourse.tile as tile
    from concourse import bacc, mybir

    f16 = mybir.dt.float16
    f32 = mybir.dt.float32
    copy_f = mybir.ActivationFunctionType.Copy
    nc = bacc.Bacc("TRN2", target_bir_lowering=False, debug=False,
                   enable_asserts=False, num_devices=N_CORES)

    k1s = sorted({k1 for k1, _, _ in sig})
    k2s = sorted({k2 for _, k2, _ in sig})
    # column offset of each expert's Toeplitz block inside the fused T tile
    toff = {}
    tcols = 0
    for k in k1s:
        toff[("s1", k)] = tcols
        tcols += k * H
    for k in k2s:
        toff[("s2", k)] = tcols
        tcols += k * H
    # column offset of each chunk inside the fused x/o1/o2 tiles
    goff = []
    xcols = 0
    for k1, k2, n in sig:
        goff.append(xcols)
        xcols += n * W

    t_d = nc.dram_tensor("t", [CPC, H, tcols], f16, kind="ExternalInput").ap()
    x_d = nc.dram_tensor("x", [CPC, H, xcols], f16, kind="ExternalInput").ap()
    o1_d = nc.dram_tensor("o1", [CPC, H, xcols], f16,
                          kind="ExternalOutput").ap()
    o2_d = nc.dram_tensor("o2", [CPC, H, xcols], f16,
                          kind="ExternalOutput").ap()

    def conv_mms(psum, tt, t0, src, s0, k, dil, n):
        """psum[:, :n*W] += conv(src at col offset s0) with T block at t0."""
        c0 = k // 2
        order = [c0] + [dw for dw in range(k) if dw != c0]
        for j, dw in enumerate(order):
            d = (dw - c0) * dil
            a = max(0, -d)
            ln = W - abs(d)
            nc.tensor.matmul(
                out=psum[:, n * a:n * (a + ln)],
                lhsT=tt[:, t0 + dw * H:t0 + (dw + 1) * H],
                rhs=src[:, s0 + n * (a + d):s0 + n * (a + d + ln)],
                start=(j == 0), stop=(j == len(order) - 1),
                skip_group_check=True)

    with tile.TileContext(nc) as tc:
        with (tc.tile_pool(name="xp", bufs=3) as xp,
              tc.tile_pool(name="o1p", bufs=3) as o1p,
              tc.tile_pool(name="o2p", bufs=3) as o2p,
              tc.tile_pool(name="tp", bufs=2) as tp,
              tc.tile_pool(name="ps", bufs=6, space="PSUM") as ps):
            pend = []
            s2_left = {}          # channel -> stage-2 chunks not yet emitted

            t1cols = sum(k * H for k in k1s)

            def emit_stage2(st):
                g, u, k2, n, o1c, o2c, tt = st
                p2 = ps.tile([128, n * W], f32, tag="ps")
                conv_mms(p2, tt, toff[("s2", k2)], o1c, goff[g], k2, DIL2, n)
                nc.scalar.activation(out=o2c[:, goff[g]:goff[g] + n * W],
                                     in_=p2[:], func=copy_f)
                if u == CPC - 1:
                    # kernel tail: store per-chunk so the last store is small
                    nc.scalar.dma_start(
                        out=o2_d[u][:, goff[g]:goff[g] + n * W],
                        in_=o2c[:, goff[g]:goff[g] + n * W])
                else:
                    s2_left[u] -= 1
                    if s2_left[u] == 0:
                        nc.scalar.dma_start(out=o2_d[u], in_=o2c[:])

            for u in range(CPC):
                tt = tp.tile([128, tcols], f16, tag="t", name=f"tt_{u}")
                if u == 0:
                    # kernel head: stage-1 weights + x land first so the PE
                    # starts ~10us earlier than one fused 1.7MB transfer
                    nc.sync.dma_start(out=tt[:, :t1cols],
                                      in_=t_d[u][:, :t1cols])
                else:
                    nc.sync.dma_start(out=tt[:], in_=t_d[u])
                xc = xp.tile([128, xcols], f16, tag="x", name=f"xc_{u}")
                nc.sync.dma_start(out=xc[:], in_=x_d[u])
                if u == 0:
                    nc.sync.dma_start(out=tt[:, t1cols:],
                                      in_=t_d[u][:, t1cols:])
                o1c = o1p.tile([128, xcols], f16, tag="o1", name=f"o1c_{u}")
                o2c = o2p.tile([128, xcols], f16, tag="o2", name=f"o2c_{u}")
                s2_left[u] = len(sig)
                for g, (k1, k2, n) in enumerate(sig):
                    p1 = ps.tile([128, n * W], f32, tag="ps")
                    conv_mms(p1, tt, toff[("s1", k1)], xc, goff[g],
                             k1, DIL1, n)
                    nc.vector.tensor_copy(
                        out=o1c[:, goff[g]:goff[g] + n * W], in_=p1[:])
                    pend.append((g, u, k2, n, o1c, o2c, tt))
                    if len(pend) > 2:
                        emit_stage2(pend.pop(0))
                    if g == len(sig) - 1:
                        nc.scalar.dma_start(out=o1_d[u], in_=o1c[:])
            while pend:
                emit_stage2(pend.pop(0))
    nc.compile()
    return nc


# ------------------------------------------------------------------- driver
def kernel(x, aw1, ab1, aw2, ab2, w1_3, b1_3, w1_5, b1_5,
           w2_7, b2_7, w2_9, b2_9, w2_11, b2_11):
    from concourse.bass_utils import run_bass_kernel_spmd

    x = np.ascontiguousarray(np.asarray(x, dtype=np.float32))
    assert x.shape == (B, C, H, W)

    idx1, idx2 = _gating(np.asarray(x), np.asarray(aw1), np.asarray(ab1),
                         np.asarray(aw2), np.asarray(ab2))
    chunks = _chunk_samples(idx1, idx2)
    sig = tuple((K1S[e1], K2S[e2], len(ss)) for e1, e2, ss in chunks)

    b1e = [np.asarray(b, np.float32) for b in (b1_3, b1_5)]
    b2e = [np.asarray(b, np.float32) for b in (b2_7, b2_9, b2_11)]

    if sig not in _PROGS:
        _PROGS[sig] = _build_program(sig)
    nc = _PROGS[sig]

    w1e = [np.asarray(w, np.float32)[:, 0] for w in (w1_3, w1_5)]
    w2e = [np.asarray(w, np.float32)[:, 0] for w in (w2_7, w2_9, w2_11)]
    k1s = sorted({K1S[e1] for e1, _, _ in chunks})
    k2s = sorted({K2S[e2] for _, e2, _ in chunks})
    e1_of = {K1S[e]: e for e in range(2)}
    e2_of = {K2S[e]: e for e in range(3)}
    tparts = [_band(w1e[e1_of[k]], DIL1) for k in k1s]
    tparts += [_band(w2e[e2_of[k]], DIL2) for k in k2s]
    tall = np.concatenate(tparts, axis=2)  # [C, H, tcols] fp16

    # fused interleaved x: per chunk [C, H, W, n] -> concat -> [C, H, xcols]
    xg = []
    for e1, e2, ss in chunks:
        xi = np.stack([x[s] for s in ss], axis=-1)
        xg.append(xi.reshape(C, H, len(ss) * W))
    xall = np.concatenate(xg, axis=2).astype(np.float16)

    in_maps = []
    for core in range(N_CORES):
        cs = slice(core * CPC, (core + 1) * CPC)
        in_maps.append({"t": tall[cs], "x": np.ascontiguousarray(xall[cs])})

    res = run_bass_kernel_spmd(nc, in_maps, list(range(N_CORES)))

    out1 = np.empty((B, C, H, W), np.float32)
    out2 = np.empty((B, C, H, W), np.float32)
    goff = np.cumsum([0] + [len(ss) * W for _, _, ss in chunks])
    for core in range(N_CORES):
        cs = slice(core * CPC, (core + 1) * CPC)
        r = res.results[core]
        o1 = r["o1"].astype(np.float32)
        o2 = r["o2"].astype(np.float32)
        for g, (e1, e2, ss) in enumerate(chunks):
            n = len(ss)
            c1 = o1[:, :, goff[g]:goff[g + 1]].reshape(CPC, H, W, n)
            c2 = o2[:, :, goff[g]:goff[g + 1]].reshape(CPC, H, W, n)
            for j, s in enumerate(ss):
                out1[s, cs] = c1[..., j]
                out2[s, cs] = c2[..., j]

    # host bias add (zero in this problem family; kept for generality).
    if any(b.any() for b in b1e) or any(b.any() for b in b2e):
        corr = {}
        for s in range(B):
            e1, e2 = int(idx1[s]), int(idx2[s])
            b1, b2 = b1e[e1], b2e[e2]
            out1[s] += b1[:, None, None]
            if (e1, e2) not in corr:
                img = np.broadcast_to(b1[:, None, None].astype(np.float64),
                                      (C, H, W))
                corr[(e1, e2)] = _host_conv_nobias(
                    img, w2e[e2].astype(np.float64), DIL2)
            out2[s] += (corr[(e1, e2)] + b2[:, None, None]).astype(np.float32)
    return out1, out2


def _host_conv_nobias(x, wk, dil):
    """x [C,H,W] fp64, wk [C,k,k]: same-padded depthwise conv, no bias."""
    k = wk.shape[-1]
    c0 = k // 2
    out = np.zeros_like(x)
    for dh in range(k):
        for dw in range(k):
            dh_, dw_ = (dh - c0) * dil, (dw - c0) * dil
            hs = slice(max(0, -dh_), H - max(0, dh_))
            ws = slice(max(0, -dw_), W - max(0, dw_))
            hs2 = slice(max(0, dh_), H - max(0, -dh_))
            ws2 = slice(max(0, dw_), W - max(0, -dw_))
            out[:, hs, ws] += wk[:, dh, dw][:, None, None] * x[:, hs2, ws2]
    return out